# revision 10
# baseline (speedup 1.0000x reference)
"""Head-sharded causal self-attention (value-residual + RMSNorm + RoPE) for 8 TRN2 cores.

Sharding: 2 heads per core (tensor parallel). Each core computes q/k/v for its
128 dims, full causal attention for its heads, and a partial c_proj output;
the host sums the 8 partial [T, D] outputs (the TP all-reduce).

v2 restructure for engine overlap:
  - Two phases: B (QKV+RMSNorm+RoPE for all T) then A (attention + c_proj),
    keeping PE continuously busy (p-state 2.4 GHz).
  - bf16 inputs prepared host-side (halves input DMA); lambda folded into
    Wv / vi on host; y partials stored bf16 (halves output DMA), host sums f32.
  - Causal diag mask folded into the score-PSUM accumulation as a -240
    strict-upper-triangle matmul (no gpsimd mask multiplies).
  - Double-buffered score PSUM so score-matmul(j+1) overlaps exp(j).
  - One exp per (s-tile, both heads): [128, 2, w] ACT op.
  - 1/Z via DVE reciprocal_approx_fast instead of ACT Ln/Exp chain.
  - Softmax without max-subtraction (RMS-normed q,k bound |scores/8| <= 8).
PSUM budget (8 banks): psc 2x[128,2,512] (4; also q/k proj in B, y pairs in A)
+ pzt 1x[128,2,512] (2; z-accum, v staging) + pwa/pwb 1x[128,512] (2; ms/rope/Z).
"""
import os
import sys

sys.path.insert(0, "/opt/trn_rl_repo")

import numpy as np

import concourse.bacc as bacc
import concourse.tile as tile
import concourse.bass as bass
from concourse import mybir
from concourse.bass_utils import run_bass_kernel_spmd

N_CORES = 8
T, D, H, HD = 2048, 1024, 16, 64
HS = H // N_CORES            # 2 heads per core
J = HS * HD                  # 128
NT = T // 128                # 16 t-tiles
NCH = T // 512               # 4 chunks
KT = D // 128                # 8 contraction tiles
F32 = mybir.dt.float32
F32R = mybir.dt.float32r
BF16 = mybir.dt.bfloat16
AF = mybir.ActivationFunctionType
OP = mybir.AluOpType
EPS = float(np.finfo(np.float32).eps)


def build_nc():
    nc = bacc.Bacc("TRN2", target_bir_lowering=False, debug=False,
                   num_devices=N_CORES)

    xT = nc.dram_tensor("xT", [D, T], BF16, kind="ExternalInput")
    wqT = nc.dram_tensor("wqT", [D, J], BF16, kind="ExternalInput")
    wkT = nc.dram_tensor("wkT", [D, J], BF16, kind="ExternalInput")
    wvT = nc.dram_tensor("wvT", [D, J], BF16, kind="ExternalInput")
    wpT = nc.dram_tensor("wpT", [J, D], BF16, kind="ExternalInput")
    vic = nc.dram_tensor("vic", [T, J], BF16, kind="ExternalInput")
    Ct = nc.dram_tensor("Ct", [J, T], BF16, kind="ExternalInput")
    St = nc.dram_tensor("St", [J, T], BF16, kind="ExternalInput")
    o2r = nc.dram_tensor("o2r", [128, 128], BF16, kind="ExternalInput")
    prm = nc.dram_tensor("prm", [128, 128], BF16, kind="ExternalInput")
    p64 = nc.dram_tensor("p64", [128, 128], F32, kind="ExternalInput")
    mneg = nc.dram_tensor("mneg", [128, 128], BF16, kind="ExternalInput")
    i128 = nc.dram_tensor("i128", [128, 128], BF16, kind="ExternalInput")
    y = nc.dram_tensor("y", [T, D], BF16, kind="ExternalOutput")

    with tile.TileContext(nc) as tc:
        with (
            tc.tile_pool(name="persist", bufs=1) as pp,
            tc.tile_pool(name="work", bufs=2) as wk,
            tc.tile_pool(name="att", bufs=3) as at,
            tc.tile_pool(name="ysb", bufs=4) as yp,
            tc.tile_pool(name="psc", bufs=2, space="PSUM") as psc,
            tc.tile_pool(name="pzt", bufs=1, space="PSUM") as pzt,
            tc.tile_pool(name="pwa", bufs=1, space="PSUM") as pwa,
            tc.tile_pool(name="pwb", bufs=1, space="PSUM") as pwb,
        ):
            # ---- persistent loads (ordered so chunk-0 compute starts early) ----
            wq = pp.tile([128, KT, J], BF16, tag="wq")
            nc.sync.dma_start(out=wq, in_=wqT.rearrange("(k p) m -> p k m", p=128))
            wk_ = pp.tile([128, KT, J], BF16, tag="wk")
            nc.sync.dma_start(out=wk_, in_=wkT.rearrange("(k p) m -> p k m", p=128))
            xts = pp.tile([128, KT, T], BF16, tag="xts")
            for c in range(NCH):
                tsl = slice(512 * c, 512 * (c + 1))
                nc.sync.dma_start(
                    out=xts[:, :, tsl],
                    in_=xT.rearrange("(k p) t -> p k t", p=128)[:, :, tsl])
            o2r_sb = pp.tile([128, 128], BF16, tag="o2r")
            nc.sync.dma_start(out=o2r_sb, in_=o2r[:, :])
            prm_sb = pp.tile([128, 128], BF16, tag="prm")
            nc.sync.dma_start(out=prm_sb, in_=prm[:, :])
            csb = pp.tile([J, T], BF16, tag="csb")
            nc.sync.dma_start(out=csb, in_=Ct[:, :])
            ssb = pp.tile([J, T], BF16, tag="ssb")
            nc.sync.dma_start(out=ssb, in_=St[:, :])
            wv = pp.tile([128, KT, J], BF16, tag="wv")
            nc.sync.dma_start(out=wv, in_=wvT.rearrange("(k p) m -> p k m", p=128))
            vicsb = pp.tile([128, NT, J], BF16, tag="vicsb")
            for c in range(NCH):
                nc.sync.dma_start(
                    out=vicsb[:, 4 * c:4 * (c + 1), :],
                    in_=vic[512 * c:512 * (c + 1), :]
                        .rearrange("(ti p) c -> p ti c", p=128))
            mneg_sb = pp.tile([128, 128], BF16, tag="mneg")
            nc.gpsimd.dma_start(out=mneg_sb, in_=mneg[:, :])
            i128_sb = pp.tile([128, 128], BF16, tag="i128")
            nc.gpsimd.dma_start(out=i128_sb, in_=i128[:, :])
            p64_sb = pp.tile([128, 128], F32, tag="p64")
            nc.gpsimd.dma_start(out=p64_sb, in_=p64[:, :])
            wp = pp.tile([128, D], BF16, tag="wp")
            nc.gpsimd.dma_start(out=wp, in_=wpT[:, :])
            eps_sb = pp.tile([128, 1], F32, tag="eps")
            nc.vector.memset(eps_sb, EPS)

            # pre-load the combined ln+exp+copy ACT table so the table-load
            # pass never needs to swap tables (saves 8x 1283ns swaps)
            nc.scalar.add_instruction(mybir.InstLoadActFuncSet(
                name="preload_act_tbl", act_func_set_id=6, ins=[], outs=[]))

            # v_aug: [v_h0 | ones64 | ones64 | v_h1] per s-tile
            vaug = pp.tile([128, NT, 4, HD], BF16, tag="vaug")
            nc.gpsimd.memset(vaug[:, :, 1:3, :], 1.0)

            qh_all = pp.tile([J, T], BF16, tag="qh")
            kh = pp.tile([J, T], BF16, tag="kh")

            # ================= Phase B: q,k,v for all chunks =================
            for c in range(NCH):
                tsl = slice(512 * c, 512 * (c + 1))

                # q (slot 0) and k (slot 1) projections
                pb = psc.tile([128, 2, 512], F32, tag="sc")
                for kk in range(KT):
                    nc.tensor.matmul(pb[:, 0, :], wq[:, kk, :], xts[:, kk, tsl],
                                     start=(kk == 0), stop=(kk == KT - 1))
                for kk in range(KT):
                    nc.tensor.matmul(pb[:, 1, :], wk_[:, kk, :], xts[:, kk, tsl],
                                     start=(kk == 0), stop=(kk == KT - 1))

                # copy to SBUF bf16, then rmsnorm: ms = head-half sum of q^2
                qb = wk.tile([128, 2, 512], BF16, tag="qb")
                nc.scalar.copy(qb[:, 0, :], pb[:, 0, :])
                nc.scalar.copy(qb[:, 1, :], pb[:, 1, :])
                q2 = wk.tile([128, 2, 512], BF16, tag="q2")
                nc.vector.tensor_tensor(q2, qb, qb, OP.mult)
                msq = pwa.tile([128, 512], F32, tag="pwa")
                nc.tensor.matmul(msq, o2r_sb, q2[:, 0, :], start=True, stop=True)
                msk = pwb.tile([128, 512], F32, tag="pwb")
                nc.tensor.matmul(msk, o2r_sb, q2[:, 1, :], start=True, stop=True)
                lnm = wk.tile([128, 2, 512], F32, tag="lnm")
                nc.scalar.activation(lnm[:, 0, :], msq, AF.Ln, bias=eps_sb,
                                     scale=1.0 / HD)
                nc.scalar.activation(lnm[:, 1, :], msk, AF.Ln, bias=eps_sb,
                                     scale=1.0 / HD)
                rq = wk.tile([128, 2, 512], BF16, tag="rq")
                nc.scalar.activation(rq, lnm, AF.Exp, bias=0.0, scale=-0.5)

                qn = wk.tile([128, 2, 512], BF16, tag="qn")
                nc.vector.tensor_tensor(qn, qb, rq, OP.mult)

                # rope: qh = qn*C + (prm @ qn)*S
                qs = pwa.tile([128, 512], F32, tag="pwa")
                nc.tensor.matmul(qs, prm_sb, qn[:, 0, :], start=True, stop=True)
                ks = pwb.tile([128, 512], F32, tag="pwb")
                nc.tensor.matmul(ks, prm_sb, qn[:, 1, :], start=True, stop=True)
                t1 = wk.tile([128, 2, 512], BF16, tag="t1")
                nc.vector.tensor_tensor(t1[:, 0, :], qn[:, 0, :], csb[:, tsl],
                                        OP.mult)
                nc.vector.tensor_tensor(t1[:, 1, :], qn[:, 1, :], csb[:, tsl],
                                        OP.mult)
                t2 = wk.tile([128, 2, 512], BF16, tag="t2")
                nc.vector.tensor_tensor(t2[:, 0, :], qs, ssb[:, tsl], OP.mult)
                nc.vector.tensor_tensor(t2[:, 1, :], ks, ssb[:, tsl], OP.mult)
                nc.vector.tensor_tensor(qh_all[:, tsl], t1[:, 0, :], t2[:, 0, :],
                                        OP.add)
                nc.vector.tensor_tensor(kh[:, tsl], t1[:, 1, :], t2[:, 1, :],
                                        OP.add)

                # v in [t, j] layout; vaug[:, st, {0,3}, :] = vic' + v
                v_ps = pzt.tile([128, 2, 512], F32, tag="zt")
                for ti in range(4):
                    st = 4 * c + ti
                    vv = v_ps[:, ti // 2, 128 * (ti % 2):128 * (ti % 2) + J]
                    for kk in range(KT):
                        nc.tensor.matmul(
                            vv, xts[:, kk, 128 * st:128 * (st + 1)], wv[:, kk, :],
                            start=(kk == 0), stop=(kk == KT - 1))
                    nc.vector.tensor_tensor(
                        vaug[:, st, 0:4:3, :],
                        vicsb[:, st, :].rearrange("p (h d) -> p h d", h=2),
                        vv.rearrange("p (h d) -> p h d", h=2),
                        OP.add)

            # ================= Phase A: attention + c_proj =================
            # y-blocks of chunk c are deferred into chunk c+1's jst stream so
            # the PE never stalls on the z-norm DVE chain at chunk boundaries.
            zt_alls = {}

            def emit_y(c, ti, split):
                y_ps = psc.tile([128, 2, 512], F32, tag="sc")
                for oc in range(2):
                    nc.tensor.matmul(y_ps[:, oc, :],
                                     zt_alls[c][:, 128 * ti:128 * (ti + 1)],
                                     wp[:, 512 * oc:512 * (oc + 1)],
                                     start=True, stop=True)
                y_sb = yp.tile([128, 2, 512], BF16, tag="ysb")
                nc.vector.tensor_copy(y_sb[:, 0, :], y_ps[:, 0, :])
                if split:
                    nc.scalar.copy(y_sb[:, 1, :], y_ps[:, 1, :])
                else:
                    nc.vector.tensor_copy(y_sb[:, 1, :], y_ps[:, 1, :])
                nc.gpsimd.dma_start(
                    out=y[512 * c + 128 * ti:512 * c + 128 * (ti + 1), :],
                    in_=y_sb.rearrange("p a b -> p (a b)"))

            for c in range(NCH):
                zt = pzt.tile([128, 2, 512], F32, tag="zt")
                n_st = 4 * (c + 1)
                for jst in range(n_st):
                    loc0 = max(0, 128 * jst - 512 * c)
                    sc = psc.tile([128, 2, 512], F32, tag="sc")
                    for h in range(HS):
                        nc.tensor.matmul(
                            sc[:, h, loc0:],
                            kh[64 * h:64 * (h + 1), 128 * jst:128 * (jst + 1)],
                            qh_all[64 * h:64 * (h + 1),
                                   512 * c + loc0:512 * (c + 1)],
                            start=True, stop=True)
                        if jst >= 4 * c:  # diagonal: add -240 strict upper tri
                            nc.tensor.matmul(
                                sc[:, h, loc0:loc0 + 128], mneg_sb, i128_sb,
                                start=False, stop=True, skip_group_check=True)
                    aT = at.tile([128, 2, 512], BF16, tag="aT")
                    if loc0 == 0:
                        nc.scalar.activation(aT, sc, AF.Exp, bias=0.0,
                                             scale=1.0 / 8.0)
                    else:
                        nc.scalar.activation(aT[:, :, loc0:], sc[:, :, loc0:],
                                             AF.Exp, bias=0.0, scale=1.0 / 8.0)
                    # deferred c_proj of the previous chunk, emitted between
                    # exp and PV so its matmuls fill the PE wait-for-exp gap
                    # (pairs keep the psc ping-pong parity intact)
                    if c > 0 and jst == 2:
                        emit_y(c - 1, 0, False)
                        emit_y(c - 1, 1, False)
                    if c > 0 and jst == 4:
                        emit_y(c - 1, 2, False)
                        emit_y(c - 1, 3, False)
                    # z matmuls: h0 lhsT=[v|ones] -> z rows 0:64, Zrep 64:128
                    #            h1 lhsT=[ones|v] -> Zrep 0:64, z rows 64:128
                    for h in range(HS):
                        nc.tensor.matmul(
                            zt[:, h, loc0:],
                            vaug[:, jst, 2 * h:2 * h + 2, :],
                            aT[:, h, loc0:],
                            start=(jst == 0), stop=(jst == n_st - 1))

                # ---- z normalization: 1/Z on DVE, partition-swap via p64 mm
                zc = at.tile([128, 512], F32, tag="zc")
                nc.vector.tensor_copy(zc[64:128, :], zt[64:128, 0, :])
                nc.vector.tensor_copy(zc[0:64, :], zt[0:64, 1, :])
                zs_ps = pwa.tile([128, 512], F32, tag="pwa")
                nc.tensor.matmul(zs_ps, p64_sb, zc, start=True, stop=True)
                rz = at.tile([128, 512], F32, tag="rz")
                nc.vector.reciprocal_approx_fast(rz, zs_ps)
                zt_all = wk.tile([128, 512], BF16, tag="zta")
                nc.vector.tensor_tensor(zt_all[0:64, :], zt[0:64, 0, :],
                                        rz[0:64, :], OP.mult)
                nc.vector.tensor_tensor(zt_all[64:128, :], zt[64:128, 1, :],
                                        rz[64:128, :], OP.mult)
                zt_alls[c] = zt_all

            # tail: last chunk's c_proj; alternate copy engines per ti so
            # DVE and ACT drain the four y-pairs in parallel
            for ti in range(4):
                emit_y(NCH - 1, ti, ti % 2 == 1)

    nc.finalize()
    return nc


def _host_prep(x, vi, Wq, Wk, Wv, Wproj, lambdas):
    import ml_dtypes
    bf16 = ml_dtypes.bfloat16
    x = np.asarray(x, np.float32)[0]
    vi = np.asarray(vi, np.float32)[0]
    Wq, Wk = np.asarray(Wq, np.float32), np.asarray(Wk, np.float32)
    lam = np.asarray(lambdas, np.float32)
    Wv = np.asarray(Wv, np.float32) * lam[0]
    vi = vi * lam[1]
    Wp = np.asarray(Wproj, np.float32)

    xT = np.ascontiguousarray(x.T).astype(bf16)
    quarter = HD // 4
    inv_freq = (1.0 / 1024.0) ** np.linspace(0.0, 1.0, quarter, dtype=np.float32)
    inv_freq = np.concatenate([inv_freq, np.zeros(quarter, np.float32)])
    th = np.arange(T, dtype=np.float32)[:, None] * inv_freq[None, :]
    cos, sin = np.cos(th).astype(np.float32), np.sin(th).astype(np.float32)
    C = np.zeros((J, T), np.float32)
    S = np.zeros((J, T), np.float32)
    for h in range(HS):
        C[h * 64:h * 64 + 32] = cos.T[:32]
        C[h * 64 + 32:h * 64 + 64] = cos.T[:32]
        S[h * 64:h * 64 + 32] = sin.T[:32]
        S[h * 64 + 32:h * 64 + 64] = -sin.T[:32]
    o2r = np.zeros((128, 128), np.float32)
    o2r[0:64, 0:64] = 1.0
    o2r[64:128, 64:128] = 1.0
    prm = np.zeros((128, 128), np.float32)
    for i in range(128):
        src = i + 32 if (i % 64) < 32 else i - 32
        prm[src, i] = 1.0
    p64 = np.zeros((128, 128), np.float32)
    for i in range(128):
        p64[(i + 64) % 128, i] = 1.0
    mneg = np.zeros((128, 128), np.float32)
    for p in range(128):
        mneg[p, p + 1:] = -240.0
    i128 = np.eye(128, dtype=np.float32)

    in_maps = []
    for c in range(N_CORES):
        rows = slice(J * c, J * (c + 1))
        in_maps.append({
            "xT": xT,
            "wqT": np.ascontiguousarray(Wq[rows, :].T).astype(bf16),
            "wkT": np.ascontiguousarray(Wk[rows, :].T).astype(bf16),
            "wvT": np.ascontiguousarray(Wv[rows, :].T).astype(bf16),
            "wpT": np.ascontiguousarray(Wp[:, rows].T).astype(bf16),
            "vic": np.ascontiguousarray(vi[:, rows]).astype(bf16),
            "Ct": C.astype(bf16), "St": S.astype(bf16),
            "o2r": o2r.astype(bf16), "prm": prm.astype(bf16),
            "p64": p64, "mneg": mneg.astype(bf16),
            "i128": i128.astype(bf16),
        })
    return in_maps


_NC = None


def kernel(x, vi, Wq, Wk, Wv, Wproj, lambdas):
    global _NC
    if _NC is None:
        _NC = build_nc()
    in_maps = _host_prep(x, vi, Wq, Wk, Wv, Wproj, lambdas)
    trace = bool(int(os.environ.get("KERNEL_TRACE", "0")))
    res = run_bass_kernel_spmd(_NC, in_maps, core_ids=list(range(N_CORES)),
                               trace=trace)
    if trace and res.exec_time_ns is not None:
        print(f"HW exec time: {res.exec_time_ns} ns")
    out = np.zeros((T, D), np.float32)
    for c in range(N_CORES):
        out += np.asarray(res.results[c]["y"], np.float32)
    return out.reshape(1, T, D)


# revision 13
# speedup vs baseline: 1.1279x; 1.1279x over previous
"""Head-sharded causal self-attention (value-residual + RMSNorm + RoPE) for 8 TRN2 cores.

Sharding: 2 heads per core (tensor parallel). Each core computes q/k/v for its
128 dims, full causal attention for its heads, and a partial c_proj output;
the host sums the 8 partial [T, D] outputs (the TP all-reduce).

v2 restructure for engine overlap:
  - Two phases: B (QKV+RMSNorm+RoPE for all T) then A (attention + c_proj),
    keeping PE continuously busy (p-state 2.4 GHz).
  - bf16 inputs prepared host-side (halves input DMA); lambda folded into
    Wv / vi on host; y partials stored bf16 (halves output DMA), host sums f32.
  - Causal diag mask folded into the score-PSUM accumulation as a -240
    strict-upper-triangle matmul (no gpsimd mask multiplies).
  - Double-buffered score PSUM so score-matmul(j+1) overlaps exp(j).
  - One exp per (s-tile, both heads): [128, 2, w] ACT op.
  - 1/Z via DVE reciprocal_approx_fast instead of ACT Ln/Exp chain.
  - Softmax without max-subtraction (RMS-normed q,k bound |scores/8| <= 8).
PSUM budget (8 banks): psc 2x[128,2,512] (4; also q/k proj in B, y pairs in A)
+ pzt 1x[128,2,512] (2; z-accum, v staging) + pwa/pwb 1x[128,512] (2; ms/rope/Z).
"""
import os
import sys

sys.path.insert(0, "/opt/trn_rl_repo")

import numpy as np

import concourse.bacc as bacc
import concourse.tile as tile
import concourse.bass as bass
from concourse import mybir
from concourse.bass_utils import run_bass_kernel_spmd

N_CORES = 8
T, D, H, HD = 2048, 1024, 16, 64
HS = H // N_CORES            # 2 heads per core
J = HS * HD                  # 128
NT = T // 128                # 16 t-tiles
NCH = T // 512               # 4 chunks
KT = D // 128                # 8 contraction tiles
F32 = mybir.dt.float32
F32R = mybir.dt.float32r
BF16 = mybir.dt.bfloat16
AF = mybir.ActivationFunctionType
OP = mybir.AluOpType
EPS = float(np.finfo(np.float32).eps)


def build_nc():
    nc = bacc.Bacc("TRN2", target_bir_lowering=False, debug=False,
                   num_devices=N_CORES)

    xT = nc.dram_tensor("xT", [D, T], BF16, kind="ExternalInput")
    wqT = nc.dram_tensor("wqT", [D, J], BF16, kind="ExternalInput")
    wkT = nc.dram_tensor("wkT", [D, J], BF16, kind="ExternalInput")
    wvT = nc.dram_tensor("wvT", [D, J], BF16, kind="ExternalInput")
    wpT = nc.dram_tensor("wpT", [J, D], BF16, kind="ExternalInput")
    vic = nc.dram_tensor("vic", [T, J], BF16, kind="ExternalInput")
    Ct = nc.dram_tensor("Ct", [J, T], BF16, kind="ExternalInput")
    St = nc.dram_tensor("St", [J, T], BF16, kind="ExternalInput")
    o2r = nc.dram_tensor("o2r", [128, 128], BF16, kind="ExternalInput")
    prm = nc.dram_tensor("prm", [128, 128], BF16, kind="ExternalInput")
    p64 = nc.dram_tensor("p64", [128, 128], F32, kind="ExternalInput")
    mneg = nc.dram_tensor("mneg", [128, 128], BF16, kind="ExternalInput")
    i128 = nc.dram_tensor("i128", [128, 128], BF16, kind="ExternalInput")
    y = nc.dram_tensor("y", [T, D], BF16, kind="ExternalOutput")

    with tile.TileContext(nc) as tc:
        with (
            tc.tile_pool(name="persist", bufs=1) as pp,
            tc.tile_pool(name="work", bufs=2) as wk,
            tc.tile_pool(name="att", bufs=4) as at,
            tc.tile_pool(name="ysb", bufs=4) as yp,
            tc.tile_pool(name="psc", bufs=2, space="PSUM") as psc,
            tc.tile_pool(name="pzt", bufs=1, space="PSUM") as pzt,
            tc.tile_pool(name="pwa", bufs=1, space="PSUM") as pwa,
            tc.tile_pool(name="pwb", bufs=1, space="PSUM") as pwb,
        ):
            # ---- persistent loads (ordered so chunk-0 compute starts early) ----
            wq = pp.tile([128, KT, J], BF16, tag="wq")
            nc.sync.dma_start(out=wq, in_=wqT.rearrange("(k p) m -> p k m", p=128))
            wk_ = pp.tile([128, KT, J], BF16, tag="wk")
            nc.sync.dma_start(out=wk_, in_=wkT.rearrange("(k p) m -> p k m", p=128))
            xts = pp.tile([128, KT, T], BF16, tag="xts")
            for c in range(NCH):
                tsl = slice(512 * c, 512 * (c + 1))
                for kh2 in range(2):
                    ks = slice(4 * kh2, 4 * kh2 + 4)
                    nc.sync.dma_start(
                        out=xts[:, ks, tsl],
                        in_=xT.rearrange("(k p) t -> p k t", p=128)[:, ks, tsl])
            o2r_sb = pp.tile([128, 128], BF16, tag="o2r")
            nc.sync.dma_start(out=o2r_sb, in_=o2r[:, :])
            prm_sb = pp.tile([128, 128], BF16, tag="prm")
            nc.sync.dma_start(out=prm_sb, in_=prm[:, :])
            csb = pp.tile([J, T], BF16, tag="csb")
            nc.sync.dma_start(out=csb, in_=Ct[:, :])
            ssb = pp.tile([J, T], BF16, tag="ssb")
            nc.sync.dma_start(out=ssb, in_=St[:, :])
            wv = pp.tile([128, KT, J], BF16, tag="wv")
            nc.sync.dma_start(out=wv, in_=wvT.rearrange("(k p) m -> p k m", p=128))
            vicsb = pp.tile([128, NT, J], BF16, tag="vicsb")
            for c in range(NCH):
                nc.sync.dma_start(
                    out=vicsb[:, 4 * c:4 * (c + 1), :],
                    in_=vic[512 * c:512 * (c + 1), :]
                        .rearrange("(ti p) c -> p ti c", p=128))
            mneg_sb = pp.tile([128, 128], BF16, tag="mneg")
            nc.gpsimd.dma_start(out=mneg_sb, in_=mneg[:, :])
            i128_sb = pp.tile([128, 128], BF16, tag="i128")
            nc.gpsimd.dma_start(out=i128_sb, in_=i128[:, :])
            p64_sb = pp.tile([128, 128], F32, tag="p64")
            nc.gpsimd.dma_start(out=p64_sb, in_=p64[:, :])
            wp = pp.tile([128, D], BF16, tag="wp")
            nc.gpsimd.dma_start(out=wp, in_=wpT[:, :])
            eps_sb = pp.tile([128, 1], F32, tag="eps")
            nc.vector.memset(eps_sb, EPS)

            # pre-load the combined ln+exp+copy ACT table so the table-load
            # pass never needs to swap tables (saves 8x 1283ns swaps)
            nc.scalar.add_instruction(mybir.InstLoadActFuncSet(
                name="preload_act_tbl", act_func_set_id=6, ins=[], outs=[]))

            # v_aug: [v_h0 | ones64 | ones64 | v_h1] per s-tile
            vaug = pp.tile([128, NT, 4, HD], BF16, tag="vaug")
            nc.gpsimd.memset(vaug[:, :, 1:3, :], 1.0)

            qh_all = pp.tile([J, T], BF16, tag="qh")
            kh = pp.tile([J, T], BF16, tag="kh")

            # ================= Phase B: q,k,v for all chunks =================
            for c in range(NCH):
                tsl = slice(512 * c, 512 * (c + 1))

                # q (slot 0) and k (slot 1) projections
                pb = psc.tile([128, 2, 512], F32, tag="sc")
                for kk in range(KT):
                    nc.tensor.matmul(pb[:, 0, :], wq[:, kk, :], xts[:, kk, tsl],
                                     start=(kk == 0), stop=(kk == KT - 1))
                for kk in range(KT):
                    nc.tensor.matmul(pb[:, 1, :], wk_[:, kk, :], xts[:, kk, tsl],
                                     start=(kk == 0), stop=(kk == KT - 1))

                # copy to SBUF bf16, then rmsnorm: ms = head-half sum of q^2
                qb = wk.tile([128, 2, 512], BF16, tag="qb")
                nc.scalar.copy(qb[:, 0, :], pb[:, 0, :])
                nc.scalar.copy(qb[:, 1, :], pb[:, 1, :])
                q2 = wk.tile([128, 2, 512], BF16, tag="q2")
                nc.vector.tensor_tensor(q2, qb, qb, OP.mult)
                msq = pwa.tile([128, 512], F32, tag="pwa")
                nc.tensor.matmul(msq, o2r_sb, q2[:, 0, :], start=True, stop=True)
                msk = pwb.tile([128, 512], F32, tag="pwb")
                nc.tensor.matmul(msk, o2r_sb, q2[:, 1, :], start=True, stop=True)
                lnm = wk.tile([128, 2, 512], F32, tag="lnm")
                nc.scalar.activation(lnm[:, 0, :], msq, AF.Ln, bias=eps_sb,
                                     scale=1.0 / HD)
                nc.scalar.activation(lnm[:, 1, :], msk, AF.Ln, bias=eps_sb,
                                     scale=1.0 / HD)
                rq = wk.tile([128, 2, 512], BF16, tag="rq")
                nc.scalar.activation(rq, lnm, AF.Exp, bias=0.0, scale=-0.5)

                qn = wk.tile([128, 2, 512], BF16, tag="qn")
                nc.vector.tensor_tensor(qn, qb, rq, OP.mult)

                # rope: qh = qn*C + (prm @ qn)*S
                qs = pwa.tile([128, 512], F32, tag="pwa")
                nc.tensor.matmul(qs, prm_sb, qn[:, 0, :], start=True, stop=True)
                ks = pwb.tile([128, 512], F32, tag="pwb")
                nc.tensor.matmul(ks, prm_sb, qn[:, 1, :], start=True, stop=True)
                t1 = wk.tile([128, 2, 512], BF16, tag="t1")
                nc.vector.tensor_tensor(t1[:, 0, :], qn[:, 0, :], csb[:, tsl],
                                        OP.mult)
                nc.vector.tensor_tensor(t1[:, 1, :], qn[:, 1, :], csb[:, tsl],
                                        OP.mult)
                t2 = wk.tile([128, 2, 512], BF16, tag="t2")
                nc.vector.tensor_tensor(t2[:, 0, :], qs, ssb[:, tsl], OP.mult)
                nc.vector.tensor_tensor(t2[:, 1, :], ks, ssb[:, tsl], OP.mult)
                nc.vector.tensor_tensor(qh_all[:, tsl], t1[:, 0, :], t2[:, 0, :],
                                        OP.add)
                nc.vector.tensor_tensor(kh[:, tsl], t1[:, 1, :], t2[:, 1, :],
                                        OP.add)

                # v in [t, j] layout; vaug[:, st, {0,3}, :] = vic' + v
                v_ps = pzt.tile([128, 2, 512], F32, tag="zt")
                for ti in range(4):
                    st = 4 * c + ti
                    vv = v_ps[:, ti // 2, 128 * (ti % 2):128 * (ti % 2) + J]
                    for kk in range(KT):
                        nc.tensor.matmul(
                            vv, xts[:, kk, 128 * st:128 * (st + 1)], wv[:, kk, :],
                            start=(kk == 0), stop=(kk == KT - 1))
                    nc.vector.tensor_tensor(
                        vaug[:, st, 0:4:3, :],
                        vicsb[:, st, :].rearrange("p (h d) -> p h d", h=2),
                        vv.rearrange("p (h d) -> p h d", h=2),
                        OP.add)

            # ================= Phase A: attention + c_proj =================
            # Software-pipelined: PV lags scores by 2 s-tiles; the previous
            # chunk's z-norm chain and c_proj are emitted inside the current
            # chunk's jst stream so the PE never head-of-line blocks on them.
            zt_alls = {}
            zts = {}

            def emit_y(c, ti, split):
                y_ps = psc.tile([128, 2, 512], F32, tag="sc")
                for oc in range(2):
                    nc.tensor.matmul(y_ps[:, oc, :],
                                     zt_alls[c][:, 128 * ti:128 * (ti + 1)],
                                     wp[:, 512 * oc:512 * (oc + 1)],
                                     start=True, stop=True)
                y_sb = yp.tile([128, 2, 512], BF16, tag="ysb")
                nc.vector.tensor_copy(y_sb[:, 0, :], y_ps[:, 0, :])
                if split:
                    nc.scalar.copy(y_sb[:, 1, :], y_ps[:, 1, :])
                else:
                    nc.vector.tensor_copy(y_sb[:, 1, :], y_ps[:, 1, :])
                nc.gpsimd.dma_start(
                    out=y[512 * c + 128 * ti:512 * c + 128 * (ti + 1), :],
                    in_=y_sb.rearrange("p a b -> p (a b)"))

            def znorm_copies(c):
                zc = at.tile([128, 512], F32, tag="zc")
                nc.vector.tensor_copy(zc[64:128, :], zts[c][64:128, 0, :])
                nc.vector.tensor_copy(zc[0:64, :], zts[c][0:64, 1, :])
                return zc

            def znorm_swap(c, zc):
                zs_ps = pwa.tile([128, 512], F32, tag="pwa")
                nc.tensor.matmul(zs_ps, p64_sb, zc, start=True, stop=True)
                rz = at.tile([128, 512], F32, tag="rz")
                nc.vector.reciprocal_approx_fast(rz, zs_ps)
                return rz

            def znorm_mults(c, rz):
                zt_all = wk.tile([128, 512], BF16, tag="zta")
                nc.vector.tensor_tensor(zt_all[0:64, :], zts[c][0:64, 0, :],
                                        rz[0:64, :], OP.mult)
                nc.vector.tensor_tensor(zt_all[64:128, :], zts[c][64:128, 1, :],
                                        rz[64:128, :], OP.mult)
                zt_alls[c] = zt_all

            for c in range(NCH):
                zt = pzt.tile([128, 2, 512], F32, tag="zt")
                zts[c] = zt
                n_st = 4 * (c + 1)
                if c > 0:
                    zc_prev = znorm_copies(c - 1)
                pvq = []  # (jst, loc0, aT) awaiting their PV matmuls

                def emit_pv(jst, loc0, aT):
                    for h in range(HS):
                        nc.tensor.matmul(
                            zt[:, h, loc0:],
                            vaug[:, jst, 2 * h:2 * h + 2, :],
                            aT[:, h, loc0:],
                            start=(jst == 0), stop=(jst == n_st - 1))

                for jst in range(n_st):
                    loc0 = max(0, 128 * jst - 512 * c)
                    sc = psc.tile([128, 2, 512], F32, tag="sc")
                    for h in range(HS):
                        nc.tensor.matmul(
                            sc[:, h, loc0:],
                            kh[64 * h:64 * (h + 1), 128 * jst:128 * (jst + 1)],
                            qh_all[64 * h:64 * (h + 1),
                                   512 * c + loc0:512 * (c + 1)],
                            start=True, stop=True)
                        if jst >= 4 * c:  # diagonal: add -240 strict upper tri
                            nc.tensor.matmul(
                                sc[:, h, loc0:loc0 + 128], mneg_sb, i128_sb,
                                start=False, stop=True, skip_group_check=True)
                    aT = at.tile([128, 2, 512], BF16, tag="aT")
                    if loc0 == 0:
                        nc.scalar.activation(aT, sc, AF.Exp, bias=0.0,
                                             scale=1.0 / 8.0)
                    else:
                        nc.scalar.activation(aT[:, :, loc0:], sc[:, :, loc0:],
                                             AF.Exp, bias=0.0, scale=1.0 / 8.0)
                    pvq.append((jst, loc0, aT))
                    if c > 0 and jst == 0:
                        rz_prev = znorm_swap(c - 1, zc_prev)
                    if c > 0 and jst == 1:
                        znorm_mults(c - 1, rz_prev)
                    if len(pvq) > 2:
                        emit_pv(*pvq.pop(0))
                    if c > 0 and jst == 4:
                        emit_y(c - 1, 0, False)
                        emit_y(c - 1, 1, False)
                    if c > 0 and jst == 6:
                        emit_y(c - 1, 2, False)
                        emit_y(c - 1, 3, False)
                while pvq:
                    emit_pv(*pvq.pop(0))

            # tail: z-norm + c_proj of the last chunk (copies split DVE/ACT)
            zc3 = znorm_copies(NCH - 1)
            rz3 = znorm_swap(NCH - 1, zc3)
            znorm_mults(NCH - 1, rz3)
            for ti in range(4):
                emit_y(NCH - 1, ti, ti % 2 == 1)

    nc.finalize()
    return nc


def _host_prep(x, vi, Wq, Wk, Wv, Wproj, lambdas):
    import ml_dtypes
    bf16 = ml_dtypes.bfloat16
    x = np.asarray(x, np.float32)[0]
    vi = np.asarray(vi, np.float32)[0]
    Wq, Wk = np.asarray(Wq, np.float32), np.asarray(Wk, np.float32)
    lam = np.asarray(lambdas, np.float32)
    Wv = np.asarray(Wv, np.float32) * lam[0]
    vi = vi * lam[1]
    Wp = np.asarray(Wproj, np.float32)

    xT = np.ascontiguousarray(x.T).astype(bf16)
    quarter = HD // 4
    inv_freq = (1.0 / 1024.0) ** np.linspace(0.0, 1.0, quarter, dtype=np.float32)
    inv_freq = np.concatenate([inv_freq, np.zeros(quarter, np.float32)])
    th = np.arange(T, dtype=np.float32)[:, None] * inv_freq[None, :]
    cos, sin = np.cos(th).astype(np.float32), np.sin(th).astype(np.float32)
    C = np.zeros((J, T), np.float32)
    S = np.zeros((J, T), np.float32)
    for h in range(HS):
        C[h * 64:h * 64 + 32] = cos.T[:32]
        C[h * 64 + 32:h * 64 + 64] = cos.T[:32]
        S[h * 64:h * 64 + 32] = sin.T[:32]
        S[h * 64 + 32:h * 64 + 64] = -sin.T[:32]
    o2r = np.zeros((128, 128), np.float32)
    o2r[0:64, 0:64] = 1.0
    o2r[64:128, 64:128] = 1.0
    prm = np.zeros((128, 128), np.float32)
    for i in range(128):
        src = i + 32 if (i % 64) < 32 else i - 32
        prm[src, i] = 1.0
    p64 = np.zeros((128, 128), np.float32)
    for i in range(128):
        p64[(i + 64) % 128, i] = 1.0
    mneg = np.zeros((128, 128), np.float32)
    for p in range(128):
        mneg[p, p + 1:] = -240.0
    i128 = np.eye(128, dtype=np.float32)

    in_maps = []
    for c in range(N_CORES):
        rows = slice(J * c, J * (c + 1))
        in_maps.append({
            "xT": xT,
            "wqT": np.ascontiguousarray(Wq[rows, :].T).astype(bf16),
            "wkT": np.ascontiguousarray(Wk[rows, :].T).astype(bf16),
            "wvT": np.ascontiguousarray(Wv[rows, :].T).astype(bf16),
            "wpT": np.ascontiguousarray(Wp[:, rows].T).astype(bf16),
            "vic": np.ascontiguousarray(vi[:, rows]).astype(bf16),
            "Ct": C.astype(bf16), "St": S.astype(bf16),
            "o2r": o2r.astype(bf16), "prm": prm.astype(bf16),
            "p64": p64, "mneg": mneg.astype(bf16),
            "i128": i128.astype(bf16),
        })
    return in_maps


_NC = None


def kernel(x, vi, Wq, Wk, Wv, Wproj, lambdas):
    global _NC
    if _NC is None:
        _NC = build_nc()
    in_maps = _host_prep(x, vi, Wq, Wk, Wv, Wproj, lambdas)
    trace = bool(int(os.environ.get("KERNEL_TRACE", "0")))
    res = run_bass_kernel_spmd(_NC, in_maps, core_ids=list(range(N_CORES)),
                               trace=trace)
    if trace and res.exec_time_ns is not None:
        print(f"HW exec time: {res.exec_time_ns} ns")
    out = np.zeros((T, D), np.float32)
    for c in range(N_CORES):
        out += np.asarray(res.results[c]["y"], np.float32)
    return out.reshape(1, T, D)
